# revision 17
# baseline (speedup 1.0000x reference)
"""BinaryDense kernel for Trainium2 (8 NeuronCores, data-parallel rows).

Computes: out[N, F] = inputs[N, D] @ w_bin[D, F]
where w_bin = 2*bernoulli(hard_sigmoid(weight)) - 1 (training mode) or
sign(weight) (eval mode), exactly matching the JAX threefry reference RNG.

Strategy:
  - w_bin (128x128) is computed host-side with JAX on CPU (threefry is
    backend-invariant, so it bit-matches the reference) and replicated.
  - The 1M rows are sharded contiguously across 8 cores (131072 rows each).
  - Per core: stream X in [128, 2048]-row chunks; PE-transpose each
    [128,128] block (contraction dim must be on partitions), copy the
    transposed block PSUM->SBUF on DVE, fp32 matmul against the resident
    w_bin, copy the result PSUM->SBUF on ACT, and DMA out.
"""

import numpy as np

N_TOTAL = 1048576
D = 128
F = 128
N_CORES = 8
ROWS_PER_CORE = N_TOTAL // N_CORES  # 131072

CHUNK_ROWS = 4096
N_CHUNKS = ROWS_PER_CORE // CHUNK_ROWS  # 32
SUBTILES = CHUNK_ROWS // 128  # 32


def _binarize_weight(weight, seed, is_training):
    """Reproduce the reference's w_bin exactly.

    Must run the *same ops on the same (default) backend* as the grading
    harness's reference: the default PRNG impl here is `rbg`, whose bit
    stream is backend-dependent, so computing this on CPU would give a
    different bernoulli draw than the reference run on the neuron backend.
    """
    import jax
    import jax.numpy as jnp

    w = jnp.asarray(np.asarray(weight, dtype=np.float32))
    if int(is_training):
        rng = jax.random.key(int(seed))
        p = jnp.clip((w + 1.0) * 0.5, 0.0, 1.0)
        w_bin = jax.random.bernoulli(rng, p=p).astype(jnp.float32) * 2.0 - 1.0
    else:
        w_bin = jnp.where(w > 0, 1.0, -1.0).astype(jnp.float32)
    return np.asarray(jax.device_get(w_bin), dtype=np.float32)


def _chunk_schedule():
    """Row counts per chunk: small prologue/epilogue chunks so the pipeline
    fills and drains quickly; big 4096-row chunks in steady state."""
    pro = [512, 512, 1024, 2048]
    epi = [2048, 1024, 512, 512]
    if ROWS_PER_CORE < sum(pro) + sum(epi) + CHUNK_ROWS:
        sched = [min(CHUNK_ROWS, ROWS_PER_CORE)]
        while sum(sched) < ROWS_PER_CORE:
            sched.append(min(CHUNK_ROWS, ROWS_PER_CORE - sum(sched)))
    else:
        body = (ROWS_PER_CORE - sum(pro) - sum(epi)) // CHUNK_ROWS
        sched = pro + [CHUNK_ROWS] * body + epi
    assert sum(sched) == ROWS_PER_CORE
    return sched


def _emit_kernel(tc, y, x, w):
    import concourse.mybir as mybir
    from concourse.masks import make_identity

    nc = tc.nc
    fp32 = mybir.dt.float32

    with (
        tc.tile_pool(name="xin", bufs=3) as x_pool,
        tc.tile_pool(name="xtp", bufs=4, space="PSUM") as xt_psum_pool,
        tc.tile_pool(name="xts", bufs=4) as xt_pool,
        tc.tile_pool(name="cp", bufs=4, space="PSUM") as c_psum_pool,
        tc.tile_pool(name="cout", bufs=3) as out_pool,
        tc.tile_pool(name="const", bufs=1) as const_pool,
    ):
        w_sbuf = const_pool.tile([D, F], fp32, name="wsb")
        nc.sync.dma_start(w_sbuf[:], w[:, :])
        ident = const_pool.tile([128, 128], fp32, name="ident")
        make_identity(nc, ident[:])

        r0 = 0
        for rows in _chunk_schedule():
            t_count = rows // 128
            # Partition p holds rows r0 + p*t_count + t (t_count consecutive
            # rows per partition -> fully contiguous per-partition DMA).
            xc = x[r0 : r0 + rows, :].rearrange("(p t) d -> p (t d)", p=128)
            yc = y[r0 : r0 + rows, :].rearrange("(p t) f -> p (t f)", p=128)
            x_tile = x_pool.tile([128, rows], fp32, tag="xin")
            nc.sync.dma_start(x_tile[:], xc)
            out_tile = out_pool.tile([128, rows], fp32, tag="cout")
            for b in range(t_count // 4):
                xt_psum = xt_psum_pool.tile([128, 512], fp32, tag="xtp")
                for i in range(4):
                    col = b * 512 + i * 128
                    nc.tensor.transpose(
                        xt_psum[:, i * 128 : (i + 1) * 128],
                        x_tile[:, col : col + 128],
                        ident[:],
                    )
                xt_sbuf = xt_pool.tile([128, 512], fp32, tag="xts")
                nc.vector.tensor_copy(xt_sbuf[:], xt_psum[:])
                c_psum = c_psum_pool.tile([128, 512], fp32, tag="cp")
                for i in range(4):
                    nc.tensor.matmul(
                        c_psum[:, i * 128 : (i + 1) * 128],
                        xt_sbuf[:, i * 128 : (i + 1) * 128],
                        w_sbuf[:],
                        start=True,
                        stop=True,
                    )
                nc.scalar.copy(out_tile[:, b * 512 : (b + 1) * 512], c_psum[:])
            # Output on the Scalar HWDGE queue so loads (Sync queue) and
            # stores ride independent DMA queues.
            nc.scalar.dma_start(yc, out_tile[:])
            r0 += rows


_NC_CACHE = None


def _build_nc():
    global _NC_CACHE
    if _NC_CACHE is not None:
        return _NC_CACHE
    import concourse.bacc as bacc
    import concourse.mybir as mybir
    import concourse.tile as tile

    nc = bacc.Bacc("TRN2", target_bir_lowering=False, debug=False)
    x = nc.dram_tensor(
        "x", [ROWS_PER_CORE, D], mybir.dt.float32, kind="ExternalInput"
    ).ap()
    w = nc.dram_tensor("w", [D, F], mybir.dt.float32, kind="ExternalInput").ap()
    y = nc.dram_tensor(
        "y", [ROWS_PER_CORE, F], mybir.dt.float32, kind="ExternalOutput"
    ).ap()
    with tile.TileContext(nc) as tc:
        _emit_kernel(tc, y, x, w)
    nc.compile()
    _NC_CACHE = nc
    return nc


def run(x_full, w_bin, trace=False, tmpdir=None):
    """Run the Bass kernel on 8 cores. Returns (out_full, BassKernelResults)."""
    from concourse.bass_utils import run_bass_kernel_spmd

    nc = _build_nc()
    in_maps = [
        {
            "x": np.ascontiguousarray(
                x_full[k * ROWS_PER_CORE : (k + 1) * ROWS_PER_CORE]
            ),
            "w": w_bin,
        }
        for k in range(N_CORES)
    ]
    res = run_bass_kernel_spmd(
        nc,
        in_maps,
        core_ids=list(range(N_CORES)),
        trace=trace,
        tmpdir=tmpdir,
    )
    out = np.concatenate([res.results[k]["y"] for k in range(N_CORES)], axis=0)
    return out, res


def kernel(**inputs):
    x_full = np.ascontiguousarray(np.asarray(inputs["inputs"], dtype=np.float32))
    w_bin = _binarize_weight(
        inputs["weight"], inputs.get("seed", 0), inputs.get("is_training", 1)
    )
    out, _ = run(x_full, w_bin)
    return out


# revision 18
# speedup vs baseline: 1.0532x; 1.0532x over previous
"""BinaryDense kernel for Trainium2 (8 NeuronCores, data-parallel rows).

Computes: out[N, F] = inputs[N, D] @ w_bin[D, F]
where w_bin = 2*bernoulli(hard_sigmoid(weight)) - 1 (training mode) or
sign(weight) (eval mode), exactly matching the JAX threefry reference RNG.

Strategy:
  - w_bin (128x128) is computed host-side with JAX on CPU (threefry is
    backend-invariant, so it bit-matches the reference) and replicated.
  - The 1M rows are sharded contiguously across 8 cores (131072 rows each).
  - Per core: stream X in [128, 2048]-row chunks; PE-transpose each
    [128,128] block (contraction dim must be on partitions), copy the
    transposed block PSUM->SBUF on DVE, fp32 matmul against the resident
    w_bin, copy the result PSUM->SBUF on ACT, and DMA out.
"""

import numpy as np

N_TOTAL = 1048576
D = 128
F = 128
N_CORES = 8
ROWS_PER_CORE = N_TOTAL // N_CORES  # 131072

CHUNK_ROWS = 4096
N_CHUNKS = ROWS_PER_CORE // CHUNK_ROWS  # 32
SUBTILES = CHUNK_ROWS // 128  # 32


def _binarize_weight(weight, seed, is_training):
    """Reproduce the reference's w_bin exactly.

    Must run the *same ops on the same (default) backend* as the grading
    harness's reference: the default PRNG impl here is `rbg`, whose bit
    stream is backend-dependent, so computing this on CPU would give a
    different bernoulli draw than the reference run on the neuron backend.
    """
    import jax
    import jax.numpy as jnp

    w = jnp.asarray(np.asarray(weight, dtype=np.float32))
    if int(is_training):
        rng = jax.random.key(int(seed))
        p = jnp.clip((w + 1.0) * 0.5, 0.0, 1.0)
        w_bin = jax.random.bernoulli(rng, p=p).astype(jnp.float32) * 2.0 - 1.0
    else:
        w_bin = jnp.where(w > 0, 1.0, -1.0).astype(jnp.float32)
    return np.asarray(jax.device_get(w_bin), dtype=np.float32)


def _chunk_schedule():
    """Row counts per chunk: small prologue/epilogue chunks so the pipeline
    fills and drains quickly; big 4096-row chunks in steady state."""
    pro = [512, 512, 1024, 2048]
    epi = [2048, 1024, 512, 512]
    if ROWS_PER_CORE < sum(pro) + sum(epi) + CHUNK_ROWS:
        sched = [min(CHUNK_ROWS, ROWS_PER_CORE)]
        while sum(sched) < ROWS_PER_CORE:
            sched.append(min(CHUNK_ROWS, ROWS_PER_CORE - sum(sched)))
    else:
        body = (ROWS_PER_CORE - sum(pro) - sum(epi)) // CHUNK_ROWS
        sched = pro + [CHUNK_ROWS] * body + epi
    assert sum(sched) == ROWS_PER_CORE
    return sched


def _emit_kernel(tc, y, x, w):
    import concourse.mybir as mybir
    from concourse.masks import make_identity

    nc = tc.nc
    fp32 = mybir.dt.float32

    with (
        tc.tile_pool(name="xin", bufs=4) as x_pool,
        tc.tile_pool(name="xtp", bufs=4, space="PSUM") as xt_psum_pool,
        tc.tile_pool(name="xts", bufs=6) as xt_pool,
        tc.tile_pool(name="cp", bufs=4, space="PSUM") as c_psum_pool,
        tc.tile_pool(name="cout", bufs=4) as out_pool,
        tc.tile_pool(name="const", bufs=1) as const_pool,
    ):
        w_sbuf = const_pool.tile([D, F], fp32, name="wsb")
        nc.sync.dma_start(w_sbuf[:], w[:, :])
        ident = const_pool.tile([128, 128], fp32, name="ident")
        make_identity(nc, ident[:])

        r0 = 0
        for rows in _chunk_schedule():
            t_count = rows // 128
            # Partition p holds rows r0 + p*t_count + t (t_count consecutive
            # rows per partition -> fully contiguous per-partition DMA).
            xc = x[r0 : r0 + rows, :].rearrange("(p t) d -> p (t d)", p=128)
            yc = y[r0 : r0 + rows, :].rearrange("(p t) f -> p (t f)", p=128)
            x_tile = x_pool.tile([128, rows], fp32, tag="xin")
            nc.sync.dma_start(x_tile[:], xc)
            out_tile = out_pool.tile([128, rows], fp32, tag="cout")
            for b in range(t_count // 4):
                xt_psum = xt_psum_pool.tile([128, 512], fp32, tag="xtp")
                for i in range(4):
                    col = b * 512 + i * 128
                    nc.tensor.transpose(
                        xt_psum[:, i * 128 : (i + 1) * 128],
                        x_tile[:, col : col + 128],
                        ident[:],
                    )
                xt_sbuf = xt_pool.tile([128, 512], fp32, tag="xts")
                nc.vector.tensor_copy(xt_sbuf[:], xt_psum[:])
                c_psum = c_psum_pool.tile([128, 512], fp32, tag="cp")
                for i in range(4):
                    nc.tensor.matmul(
                        c_psum[:, i * 128 : (i + 1) * 128],
                        xt_sbuf[:, i * 128 : (i + 1) * 128],
                        w_sbuf[:],
                        start=True,
                        stop=True,
                    )
                nc.scalar.copy(out_tile[:, b * 512 : (b + 1) * 512], c_psum[:])
            # Output on the Scalar HWDGE queue so loads (Sync queue) and
            # stores ride independent DMA queues.
            nc.scalar.dma_start(yc, out_tile[:])
            r0 += rows


_NC_CACHE = None


def _build_nc():
    global _NC_CACHE
    if _NC_CACHE is not None:
        return _NC_CACHE
    import concourse.bacc as bacc
    import concourse.mybir as mybir
    import concourse.tile as tile

    nc = bacc.Bacc("TRN2", target_bir_lowering=False, debug=False)
    x = nc.dram_tensor(
        "x", [ROWS_PER_CORE, D], mybir.dt.float32, kind="ExternalInput"
    ).ap()
    w = nc.dram_tensor("w", [D, F], mybir.dt.float32, kind="ExternalInput").ap()
    y = nc.dram_tensor(
        "y", [ROWS_PER_CORE, F], mybir.dt.float32, kind="ExternalOutput"
    ).ap()
    with tile.TileContext(nc) as tc:
        _emit_kernel(tc, y, x, w)
    nc.compile()
    _NC_CACHE = nc
    return nc


def run(x_full, w_bin, trace=False, tmpdir=None):
    """Run the Bass kernel on 8 cores. Returns (out_full, BassKernelResults)."""
    from concourse.bass_utils import run_bass_kernel_spmd

    nc = _build_nc()
    in_maps = [
        {
            "x": np.ascontiguousarray(
                x_full[k * ROWS_PER_CORE : (k + 1) * ROWS_PER_CORE]
            ),
            "w": w_bin,
        }
        for k in range(N_CORES)
    ]
    res = run_bass_kernel_spmd(
        nc,
        in_maps,
        core_ids=list(range(N_CORES)),
        trace=trace,
        tmpdir=tmpdir,
    )
    out = np.concatenate([res.results[k]["y"] for k in range(N_CORES)], axis=0)
    return out, res


def kernel(**inputs):
    x_full = np.ascontiguousarray(np.asarray(inputs["inputs"], dtype=np.float32))
    w_bin = _binarize_weight(
        inputs["weight"], inputs.get("seed", 0), inputs.get("is_training", 1)
    )
    out, _ = run(x_full, w_bin)
    return out
